# revision 21
# baseline (speedup 1.0000x reference)
"""Self-attention (SAGAN-style, spectral-normalized 1x1 convs) on 8 TRN2 cores.

Contract: kernel(**inputs) takes the FULL unsharded inputs
(x [8,512,64,64], weights, power-iteration u vectors, gamma) and returns
the FULL output [8,512,64,64] (float32).

Sharding: data-parallel over batch B=8 -> one batch element per core.
Each core runs the complete attention block for its element; no
collectives are needed.

Per-core math (C=512, HW=4096, M=HW/4=1024):
    theta = sn(w_theta) @ x          [64, 4096]
    phi   = maxpool2(sn(w_phi) @ x)  [64, 1024]
    g     = maxpool2(sn(w_g)   @ x)  [256, 1024]
    sT[m,n] = sum_c phi[c,m] theta[c,n]
    beta  = softmax over m  (exp without max-subtraction: logits span
            ~+-51, safe in fp32/bf16; normalization applied to o)
    o     = (g @ exp(sT)) * (1/sum)  [256, 4096]
    out   = gamma * (sn(w_o) @ o) + x

Precision plan (measured 9.4e-3 output rel err vs the 2e-2 gate):
 - projections + logits fp16 (x16 from host, theta, phi, wg), bf16 E/g
 - o stored e4m3; out-projection fp8 e4m3 DoubleRow (k=256 per matmul)
   with wo pre-scaled by 512*gamma on the host; the 1/512 rides in the
   scale slot of the residual ops; fp32 PSUM throughout
 - residual + output fp16 (host upcasts to fp32)

Schedule notes (from perfetto/ntff analysis of earlier versions):
 - FD=512 matmuls pipeline at ~216ns start-to-start (incl. implicit
   LDWEIGHTS), so wall time ~ matmul slot count. Counts per core:
   theta/phi 32, g 64, transposes 16, per block: 4 packed sT pairs,
   2 accumulating ones-matmuls, 16 o, 4 fp8-DR out-proj.
 - input = ONE 4MB fp16 x copy (an fp8 x copy for a DoubleRow g-proj
   was tried and dropped: input DMA bandwidth paces the head, PE has
   slack there, so fp16 g-matmuls are free and more accurate); x chunks
   alternate sync/scalar DMA queues, weights ride gpsimd.
 - ~14 junk warmup matmuls bridge the DMA head so the HAM clock-gate
   never re-throttles the PE to 1.2GHz (cost ~25us in earlier runs).
 - softmax sums: 2-level DVE tree + two accumulating ones-matmuls
   (replaces 8 matmuls/block); reciprocal_approx_fast (the exact
   reciprocal measures 6.4 cyc/elem = 27us/kernel); sums sit between
   the two o-matmul halves so the o-PSUM rotation never stalls.
 - out-projection of block nb-1 is emitted split: PE matmuls + 2 DVE
   fused residuals (scalar_tensor_tensor) early (fills the exp-pipeline
   wait), ACT copy-scale + GPSIMD adds + single out-DMA after the sT
   pairs so ACT's exp stream is never interrupted.

PE->PE self-waits are stripped (PE->PSUM write port is FIFO) and bacc's
generate_event_semaphores legalizes the 1-wait ISA limit.

The spectral-norm power-iteration only involves [1,64]x[64,512]
matvecs, so it runs on the host in float32; gamma is folded into w_o.
"""

import numpy as np

B, C, H, W = 8, 512, 64, 64
HW = H * W            # 4096
M = HW // 4           # 1024 (pooled spatial)
C8 = C // 8           # 64
C2 = C // 2           # 256
P = 128               # SBUF partitions
KC = C // P           # 4 k-chunks for C-contraction
FB = 512              # free-dim block
NB = HW // FB         # 8 n-blocks
MC = M // P           # 8 m-chunks
WO_SCALE = 512.0      # host wo scale (e4m3 normal range), cancelled in
                      # the residual ops' scale slots
EPS = 1e-12

_CACHE = {}


def _sn(w, u):
    """Host-side spectral norm (eval-mode power iteration), float32."""
    w = np.asarray(w, np.float32)
    u = np.asarray(u, np.float32)
    v = u @ w
    v = v / max(np.float32(np.linalg.norm(v)), np.float32(EPS))
    u2 = v @ w.T
    u2 = u2 / max(np.float32(np.linalg.norm(u2)), np.float32(EPS))
    sv = np.float32((v @ w.T @ u2.T)[0, 0])
    return w / sv


def _strip_pe_self_waits(nc):
    """Remove S[PE]-waits from PE matmuls: PE->PE deps are ordered by the
    engine queue + FIFO PSUM write port, and matmuls only have one ISA
    wait slot."""
    import concourse.mybir as mybir

    for f in nc.m.functions:
        for blk in f.blocks:
            for inst in blk.instructions:
                if not isinstance(inst, mybir.InstMatmult):
                    continue
                si = inst.sync_info
                kept = [w for w in si.on_wait
                        if not (w.ant_name or "").startswith("PE_")]
                if len(kept) != len(si.on_wait):
                    si.on_wait = kept
                    inst.sync_info = si


def _build_nc():
    import concourse.bass as bass
    import concourse.mybir as mybir
    import concourse.tile as tile
    from concourse import bacc
    from concourse.masks import make_identity

    fp32 = mybir.dt.float32
    fp16 = mybir.dt.float16
    bf16 = mybir.dt.bfloat16
    fp8 = mybir.dt.float8e4
    DR = mybir.MatmulPerfMode.DoubleRow
    Exp = mybir.ActivationFunctionType.Exp
    mult = mybir.AluOpType.mult
    add = mybir.AluOpType.add
    mx = mybir.AluOpType.max

    nc = bacc.Bacc()
    # weights arrive host-pretransposed to [p, ...] layouts so their DMAs
    # are contiguous per-partition lines on the HARDWARE dynamic queues
    # (the gpsimd software queue starts ~4us late and paced the old head)
    x16_d = nc.dram_tensor("x16", [C, HW], fp16, kind="ExternalInput").ap()
    wtp_d = nc.dram_tensor("wtp", [P, KC * P], fp16, kind="ExternalInput").ap()
    wg_d = nc.dram_tensor("wg", [P, KC * C2], fp16, kind="ExternalInput").ap()
    wo_d = nc.dram_tensor("wo", [P, 2 * C], fp8, kind="ExternalInput").ap()
    out_d = nc.dram_tensor("out", [C, HW], fp16, kind="ExternalOutput").ap()

    x16_r = x16_d.rearrange("(kc p) n -> p kc n", p=P)
    out_r = out_d.rearrange("(ig p) n -> p ig n", p=P)

    with tile.TileContext(nc) as tc:
        with tc.tile_pool(name="sb", bufs=1) as sb:
            # ---- persistent tiles ----
            x16 = sb.tile([P, KC, HW], fp16)
            wtp = sb.tile([P, KC, P], fp16)
            wg2 = sb.tile([P, KC, C2], fp16)
            wo8 = sb.tile([P, 2, C], fp8)
            theta_sb = sb.tile([P, HW], fp16)             # rows 64:128 duplicate
            phi2 = sb.tile([P, NB, 4, 32], fp16)          # rows 64:128 duplicate
            g2 = sb.tile([P, 2, M], bf16)                 # pooled, cg-major
            gT_sb = sb.tile([P, MC, C2], bf16)            # [m-part, mc, c]
            identity = sb.tile([P, P], bf16)
            ones_mat = sb.tile([P, P], bf16)

            phi_flat = phi2.rearrange("p a b c -> p (a b c)")
            g4 = g2.rearrange("p cg (fb h2 w2) -> p cg fb h2 w2", h2=4, w2=32)

            # ---- constants + HAM warmup ----
            nc.vector.memset(ones_mat, 1.0)
            with tc.tile_pool(name="psW", bufs=1, space="PSUM") as psW:
                # junk matmuls keep the PE busy during the DMA head so the
                # HAM clock-gate opens before the real matmul stream
                wps = psW.tile([P, P], fp32, tag="warm")
                for _ in range(20):
                    nc.tensor.matmul(wps, lhsT=ones_mat, rhs=ones_mat,
                                     start=True, stop=True)
            ident_raw = sb.tile([P, P], fp32)
            make_identity(nc, ident_raw)
            nc.scalar.copy(identity, ident_raw)

            # ---- input DMAs on the two hardware queues ----
            # Each queue sustains ~215GB/s (~430 combined); consumption
            # order is column order, so alternate column segments between
            # queues, with small leading segments so the first projection
            # matmul (needs wtp + cols 0:512) can start ~10.3us.  Weights
            # lead their queue: wtp ~8.9, wg ~11.3 (needed ~12), wo ~10.7
            # (needed ~45).
            nc.scalar.dma_start(x16[:, :, 0:256], x16_r[:, :, 0:256])
            nc.sync.dma_start(wtp, wtp_d.rearrange("p (kc i) -> p kc i", kc=KC))
            nc.scalar.dma_start(wo8, wo_d.rearrange("p (cg i) -> p cg i", cg=2))
            nc.sync.dma_start(x16[:, :, 256:512], x16_r[:, :, 256:512])
            nc.sync.dma_start(wg2, wg_d.rearrange("p (kc i) -> p kc i", kc=KC))
            segs = [(512, 768, nc.scalar), (768, 1280, nc.sync),
                    (1280, 1792, nc.scalar), (1792, 2304, nc.sync),
                    (2304, 2816, nc.scalar), (2816, 3328, nc.sync),
                    (3328, 3840, nc.scalar), (3840, 4096, nc.sync)]
            for lo, hi, eng in segs:
                eng.dma_start(x16[:, :, lo:hi], x16_r[:, :, lo:hi])

            # ---------- projections ----------
            # block 0's attention front (sT pairs -> exp -> sums -> recip)
            # is streamed through this phase: pair mc2 rides at the end of
            # fb2==mc2 (whose phi chunks it consumes), so the proj->attention
            # transition has no PE bubble (which used to trip the HAM
            # throttle back to 1.2GHz)
            expT0 = sb.tile([P, MC, FB], bf16, tag="expT", bufs=3)
            part0 = sb.tile([P, 4, FB], bf16, tag="part", bufs=2)
            with (
                tc.tile_pool(name="psA", bufs=3, space="PSUM") as psA,
                tc.tile_pool(name="psT", bufs=2, space="PSUM") as psT,
            ):
                def _f0_pair(mc2):
                    ps = psA.tile([P, 2, FB], fp32, tag="proj", name="st0")
                    nc.tensor.matmul(
                        ps[:, 0, :],
                        lhsT=phi_flat[:C8, (2 * mc2) * P:(2 * mc2 + 1) * P],
                        rhs=theta_sb[:C8, 0:FB],
                        start=True, stop=True, tile_position=(0, 0),
                    )
                    nc.tensor.matmul(
                        ps[:, 1, :],
                        lhsT=phi_flat[C8:, (2 * mc2 + 1) * P:(2 * mc2 + 2) * P],
                        rhs=theta_sb[C8:, 0:FB],
                        start=True, stop=True, tile_position=(64, 0),
                    )
                    nc.scalar.activation(
                        expT0[:, 2 * mc2:2 * mc2 + 2, :].rearrange("p a b -> p (a b)"),
                        ps.rearrange("p a b -> p (a b)"), Exp,
                    )

                for fb2 in range(NB // 2):
                    # block-0 sT pair for the PREVIOUS fb2's phi chunks: its
                    # dependencies (phi pools on DVE) are long done, so it
                    # doesn't stall the PE stream
                    if fb2 > 0:
                        _f0_pair(fb2 - 1)
                    # fused theta+phi projection: lhsT = [wt | wp] gives
                    # theta on out-partitions 0:64, phi on 64:128; both
                    # duplicated onto partitions 64:128 for sT row-packing
                    ps = psA.tile([P, 2, FB], fp32, tag="proj", name="ps")
                    for half in range(2):
                        for kc in range(KC):
                            nc.tensor.matmul(
                                ps[:, half, :],
                                lhsT=wtp[:, kc, :],
                                rhs=x16[:, kc, (2 * fb2 + half) * FB:(2 * fb2 + half + 1) * FB],
                                start=(kc == 0), stop=(kc == KC - 1),
                            )
                    th = ps[:C8].rearrange("p a b -> p (a b)")
                    thsl = slice(2 * fb2 * FB, (2 * fb2 + 2) * FB)
                    nc.scalar.copy(theta_sb[:C8, thsl], th)
                    # duplication for the sT row-packing rides gpsimd (idle
                    # during projections) instead of the busy DVE
                    nc.gpsimd.tensor_copy(theta_sb[C8:, thsl],
                                          theta_sb[:C8, thsl])
                    v = ps[C8:].rearrange("p fb (h2 hr w2 wr) -> p fb h2 hr w2 wr",
                                          hr=2, w2=32, wr=2)
                    dst = phi2[:C8, 2 * fb2:2 * fb2 + 2]   # [64, 2, 4, 32]
                    nc.vector.tensor_copy(dst, v[:, :, :, 0, :, 0])
                    nc.vector.tensor_tensor(dst, dst, v[:, :, :, 0, :, 1], mx)
                    nc.vector.tensor_tensor(dst, dst, v[:, :, :, 1, :, 0], mx)
                    nc.vector.tensor_tensor(dst, dst, v[:, :, :, 1, :, 1], mx)
                    nc.vector.tensor_copy(phi2[C8:, 2 * fb2:2 * fb2 + 2],
                                          phi2[:C8, 2 * fb2:2 * fb2 + 2])

                    # g projection + maxpool on the same x16 columns, then
                    # transpose this fb2's two m-chunks while they're hot
                    for cg in range(2):
                        ps = psA.tile([P, 2, FB], fp32, tag="proj", name="psg")
                        for half in range(2):
                            for kc in range(KC):
                                nc.tensor.matmul(
                                    ps[:, half, :],
                                    lhsT=wg2[:, kc, cg * P:(cg + 1) * P],
                                    rhs=x16[:, kc, (2 * fb2 + half) * FB:(2 * fb2 + half + 1) * FB],
                                    start=(kc == 0), stop=(kc == KC - 1),
                                )
                        v = ps.rearrange("p fb (h2 hr w2 wr) -> p fb h2 hr w2 wr",
                                         hr=2, w2=32, wr=2)
                        dst = g4[:, cg, 2 * fb2:2 * fb2 + 2]
                        nc.vector.tensor_copy(dst, v[:, :, :, 0, :, 0])
                        nc.vector.tensor_tensor(dst, dst, v[:, :, :, 0, :, 1], mx)
                        nc.vector.tensor_tensor(dst, dst, v[:, :, :, 1, :, 0], mx)
                        nc.vector.tensor_tensor(dst, dst, v[:, :, :, 1, :, 1], mx)
                    for mc in (2 * fb2, 2 * fb2 + 1):
                        pt = psT.tile([P, 2, P], bf16, tag="tr")
                        for cg in range(2):
                            nc.tensor.transpose(
                                pt[:, cg, :], g2[:, cg, mc * P:(mc + 1) * P],
                                identity,
                            )
                        nc.scalar.copy(gT_sb[:, mc, :],
                                       pt.rearrange("p a b -> p (a b)"))

                # finish block 0's front: last pair + L1 adds (sums + recip
                # stay in the attention loop); by now fb2=3's phi pools are
                # done so nothing here stalls the PE
                _f0_pair(3)
                nc.vector.tensor_tensor(
                    part0[:, 0:2, :].rearrange("p a b -> p (a b)"),
                    expT0[:, 0:2, :].rearrange("p a b -> p (a b)"),
                    expT0[:, 4:6, :].rearrange("p a b -> p (a b)"), add)
                nc.vector.tensor_tensor(
                    part0[:, 2:4, :].rearrange("p a b -> p (a b)"),
                    expT0[:, 2:4, :].rearrange("p a b -> p (a b)"),
                    expT0[:, 6:8, :].rearrange("p a b -> p (a b)"), add)

            # ---------- attention ----------
            with (
                tc.tile_pool(name="psS", bufs=2, space="PSUM") as psS,
                tc.tile_pool(name="psO", bufs=2, space="PSUM") as psO,
                tc.tile_pool(name="psO2", bufs=2, space="PSUM") as psO2,
            ):
                pending = []

                def _emit_o2_early(item):
                    """Out-projection matmuls of block jnb (4 fp8-DR, k=256)
                    + the 2 DVE fused residuals. PE work lands early to fill
                    the exp-pipeline wait of the current block."""
                    jnb, jo8 = item
                    jsl = slice(jnb * FB, (jnb + 1) * FB)
                    ot4 = sb.tile([P, 4, FB], fp16, tag="out", bufs=2,
                                  name="ot4")
                    o2s = []
                    for ig in range(4):
                        o2 = psO2.tile([P, FB], fp32, tag="o2", name="o2")
                        nc.tensor.matmul(
                            o2,
                            lhsT=wo8[:, :, ig * P:(ig + 1) * P],
                            rhs=jo8,
                            start=True, stop=True, perf_mode=DR,
                        )
                        if ig < 2:
                            # DVE: (o2 * 1/WO_SCALE) + x16 in one op
                            nc.vector.scalar_tensor_tensor(
                                ot4[:, ig, :], o2, 1.0 / WO_SCALE,
                                x16[:, ig, jsl], op0=mult, op1=add)
                        else:
                            o2s.append(o2)
                    return (jnb, ot4, o2s)

                def _emit_o2_late(item):
                    """ACT copy-with-scale + GPSIMD residual adds for ig 2,3
                    and the single out-DMA; emitted after the sT pairs so
                    ACT's exp stream is not interrupted."""
                    jnb, ot4, o2s = item
                    jsl = slice(jnb * FB, (jnb + 1) * FB)
                    for ig, o2 in zip((2, 3), o2s):
                        st = sb.tile([P, FB], fp32, tag="stage", bufs=4,
                                     name="st")
                        nc.scalar.mul(st, o2, 1.0 / WO_SCALE)
                        nc.gpsimd.tensor_tensor(ot4[:, ig, :], st,
                                                x16[:, ig, jsl], add)
                    nc.sync.dma_start(out_r[:, :, jsl], ot4)

                for nb in range(NB):
                    nsl = slice(nb * FB, (nb + 1) * FB)
                    if nb == 0:
                        # front was streamed through the projection phase
                        expT, part = expT0, part0
                    else:
                        # sT[m, n] = sum_c phi[c, m] theta[c, n]: k=64 pairs
                        # run concurrently in disjoint PE row-halves
                        expT = sb.tile([P, MC, FB], bf16, tag="expT", bufs=3)

                        def _st_pair(mc2):
                            ps = psS.tile([P, 2, FB], fp32, tag="sT")
                            nc.tensor.matmul(
                                ps[:, 0, :],
                                lhsT=phi_flat[:C8, (2 * mc2) * P:(2 * mc2 + 1) * P],
                                rhs=theta_sb[:C8, nsl],
                                start=True, stop=True, tile_position=(0, 0),
                            )
                            nc.tensor.matmul(
                                ps[:, 1, :],
                                lhsT=phi_flat[C8:, (2 * mc2 + 1) * P:(2 * mc2 + 2) * P],
                                rhs=theta_sb[C8:, nsl],
                                start=True, stop=True, tile_position=(64, 0),
                            )
                            nc.scalar.activation(
                                expT[:, 2 * mc2:2 * mc2 + 2, :].rearrange("p a b -> p (a b)"),
                                ps.rearrange("p a b -> p (a b)"), Exp,
                            )

                        part = sb.tile([P, 4, FB], bf16, tag="part", bufs=2)
                        _st_pair(0)
                        _st_pair(1)
                        # previous block's out-proj matmuls fill the exp wait
                        held = _emit_o2_early(pending.pop(0)) if pending else None
                        _st_pair(2)
                        # sum over m, level 1: two DVE adds, each ready as
                        # soon as its pair of exps lands (a: exps 0+2,
                        # b: exps 1+3), then FOUR accumulating ones-matmuls
                        # finish the m-sum on the PE
                        nc.vector.tensor_tensor(
                            part[:, 0:2, :].rearrange("p a b -> p (a b)"),
                            expT[:, 0:2, :].rearrange("p a b -> p (a b)"),
                            expT[:, 4:6, :].rearrange("p a b -> p (a b)"), add)
                        _st_pair(3)
                        if held is not None:
                            _emit_o2_late(held)
                        nc.vector.tensor_tensor(
                            part[:, 2:4, :].rearrange("p a b -> p (a b)"),
                            expT[:, 2:4, :].rearrange("p a b -> p (a b)"),
                            expT[:, 6:8, :].rearrange("p a b -> p (a b)"), add)

                    # o[c, n] = sum_m gT[m, c] expT[m, n]; the sum-matmuls
                    # sit between the two cg halves so the recip is ready
                    # when the first half finishes accumulating
                    o8_sb = sb.tile([P, 2, FB], fp8, tag="o8", bufs=2)
                    o_ps0 = psO.tile([P, FB], fp32, tag="o_ps", name="o0")
                    for mc in range(MC):
                        nc.tensor.matmul(
                            o_ps0,
                            lhsT=gT_sb[:, mc, 0:P],
                            rhs=expT[:, mc, :],
                            start=(mc == 0), stop=(mc == MC - 1),
                        )
                    sum_ps = psS.tile([P, 2, FB], fp32, tag="sT",
                                      name="sum_ps")[:, 0, :]
                    for j in range(4):
                        nc.tensor.matmul(sum_ps, lhsT=ones_mat,
                                         rhs=part[:, j, :],
                                         start=(j == 0), stop=(j == 3))
                    recipb = sb.tile([P, FB], fp32, tag="recipb", bufs=2)
                    nc.vector.reciprocal_approx_fast(recipb, sum_ps)
                    o_ps1 = psO.tile([P, FB], fp32, tag="o_ps", name="o1")
                    for mc in range(MC):
                        nc.tensor.matmul(
                            o_ps1,
                            lhsT=gT_sb[:, mc, P:C2],
                            rhs=expT[:, mc, :],
                            start=(mc == 0), stop=(mc == MC - 1),
                        )
                    nc.vector.tensor_tensor(o8_sb[:, 0, :], o_ps0, recipb, mult)
                    nc.vector.tensor_tensor(o8_sb[:, 1, :], o_ps1, recipb, mult)

                    pending.append((nb, o8_sb))

                # final block: no next block's exps to protect, so all four
                # residuals ride the (now idle) DVE directly from PSUM and
                # the out-DMA is split per half so rows fly as they finish
                jnb, jo8 = pending.pop(0)
                jsl = slice(jnb * FB, (jnb + 1) * FB)
                ot4 = sb.tile([P, 4, FB], fp16, tag="out", bufs=2, name="ot4")
                for ig in range(4):
                    o2 = psO2.tile([P, FB], fp32, tag="o2", name="o2")
                    nc.tensor.matmul(
                        o2,
                        lhsT=wo8[:, :, ig * P:(ig + 1) * P],
                        rhs=jo8,
                        start=True, stop=True, perf_mode=DR,
                    )
                    nc.vector.scalar_tensor_tensor(
                        ot4[:, ig, :], o2, 1.0 / WO_SCALE,
                        x16[:, ig, jsl], op0=mult, op1=add)
                    if ig == 1:
                        nc.sync.dma_start(out_r[:, 0:2, jsl], ot4[:, 0:2, :])
                nc.sync.dma_start(out_r[:, 2:4, jsl], ot4[:, 2:4, :])

    _strip_pe_self_waits(nc)
    nc.compile()
    return nc


def _get_nc():
    if "nc" not in _CACHE:
        _CACHE["nc"] = _build_nc()
    return _CACHE["nc"]


def make_in_maps(x, w_theta, w_phi, w_g, w_o, u_theta, u_phi, u_g, u_o, gamma):
    import ml_dtypes

    e4 = ml_dtypes.float8_e4m3
    wt = _sn(w_theta, u_theta).T                                  # [512, 64]
    wp = _sn(w_phi, u_phi).T                                      # [512, 64]
    wtp = np.concatenate([wt, wp], axis=1).astype(np.float16)     # [512, 128]
    wg = _sn(w_g, u_g).T.astype(np.float16)                       # [512, 256]
    wo = (WO_SCALE * np.float32(np.asarray(gamma, np.float32))
          * _sn(w_o, u_o).T).astype(e4)                           # [256, 512]
    # pre-transpose to [p, kc, i] so the device DMA is a contiguous
    # per-partition copy on the hardware dynamic queues
    wtp = np.ascontiguousarray(
        wtp.reshape(KC, P, P).transpose(1, 0, 2).reshape(P, KC * P))
    wg = np.ascontiguousarray(
        wg.reshape(KC, P, C2).transpose(1, 0, 2).reshape(P, KC * C2))
    wo = np.ascontiguousarray(
        wo.reshape(2, P, C).transpose(1, 0, 2).reshape(P, 2 * C))
    xf = np.asarray(x, np.float32).reshape(B, C, HW)
    x16 = xf.astype(np.float16)
    return [
        {"x16": np.ascontiguousarray(x16[i]),
         "wtp": wtp, "wg": wg, "wo": wo}
        for i in range(B)
    ]


def kernel(x, w_theta, w_phi, w_g, w_o, u_theta, u_phi, u_g, u_o, gamma):
    from concourse.bass_utils import run_bass_kernel_spmd

    in_maps = make_in_maps(
        x, w_theta, w_phi, w_g, w_o, u_theta, u_phi, u_g, u_o, gamma
    )
    nc = _get_nc()
    res = run_bass_kernel_spmd(nc, in_maps, core_ids=list(range(B)))
    out = np.stack([np.asarray(r["out"], np.float32) for r in res.results],
                   axis=0)
    return out.reshape(B, C, H, W)



# revision 30
# speedup vs baseline: 1.0107x; 1.0107x over previous
"""Self-attention (SAGAN-style, spectral-normalized 1x1 convs) on 8 TRN2 cores.

Contract: kernel(**inputs) takes the FULL unsharded inputs
(x [8,512,64,64], weights, power-iteration u vectors, gamma) and returns
the FULL output [8,512,64,64] (float32).

Sharding: data-parallel over batch B=8 -> one batch element per core.
Each core runs the complete attention block for its element; no
collectives are needed.

Per-core math (C=512, HW=4096, M=HW/4=1024):
    theta = sn(w_theta) @ x          [64, 4096]
    phi   = maxpool2(sn(w_phi) @ x)  [64, 1024]
    g     = maxpool2(sn(w_g)   @ x)  [256, 1024]
    sT[m,n] = sum_c phi[c,m] theta[c,n]
    beta  = softmax over m  (exp without max-subtraction: logits span
            ~+-51, safe in fp32/bf16; normalization applied to o)
    o     = (g @ exp(sT)) * (1/sum)  [256, 4096]
    out   = gamma * (sn(w_o) @ o) + x

Precision plan (measured 9.4e-3 output rel err vs the 2e-2 gate):
 - projections + logits fp16 (x16 from host, theta, phi, wg), bf16 E/g
 - o stored e4m3; out-projection fp8 e4m3 DoubleRow (k=256 per matmul)
   with wo pre-scaled by 512*gamma on the host; the 1/512 rides in the
   scale slot of the residual ops; fp32 PSUM throughout
 - residual + output fp16 (host upcasts to fp32)

Schedule notes (from perfetto/ntff analysis of earlier versions):
 - FD=512 matmuls pipeline at ~216ns start-to-start (incl. implicit
   LDWEIGHTS), so wall time ~ matmul slot count. Counts per core:
   theta/phi 32, g 64, transposes 16, per block: 4 packed sT pairs,
   2 accumulating ones-matmuls, 16 o, 4 fp8-DR out-proj.
 - input = ONE 4MB fp16 x copy (an fp8 x copy for a DoubleRow g-proj
   was tried and dropped: input DMA bandwidth paces the head, PE has
   slack there, so fp16 g-matmuls are free and more accurate); x chunks
   alternate sync/scalar DMA queues, weights ride gpsimd.
 - ~14 junk warmup matmuls bridge the DMA head so the HAM clock-gate
   never re-throttles the PE to 1.2GHz (cost ~25us in earlier runs).
 - softmax sums: 2-level DVE tree + two accumulating ones-matmuls
   (replaces 8 matmuls/block); reciprocal_approx_fast (the exact
   reciprocal measures 6.4 cyc/elem = 27us/kernel); sums sit between
   the two o-matmul halves so the o-PSUM rotation never stalls.
 - out-projection of block nb-1 is emitted split: PE matmuls + 2 DVE
   fused residuals (scalar_tensor_tensor) early (fills the exp-pipeline
   wait), ACT copy-scale + GPSIMD adds + single out-DMA after the sT
   pairs so ACT's exp stream is never interrupted.

PE->PE self-waits are stripped (PE->PSUM write port is FIFO) and bacc's
generate_event_semaphores legalizes the 1-wait ISA limit.

The spectral-norm power-iteration only involves [1,64]x[64,512]
matvecs, so it runs on the host in float32; gamma is folded into w_o.
"""

import numpy as np

B, C, H, W = 8, 512, 64, 64
HW = H * W            # 4096
M = HW // 4           # 1024 (pooled spatial)
C8 = C // 8           # 64
C2 = C // 2           # 256
P = 128               # SBUF partitions
KC = C // P           # 4 k-chunks for C-contraction
FB = 512              # free-dim block
NB = HW // FB         # 8 n-blocks
MC = M // P           # 8 m-chunks
WO_SCALE = 512.0      # host wo scale (e4m3 normal range), cancelled in
                      # the residual ops' scale slots
EPS = 1e-12

_CACHE = {}


def _sn(w, u):
    """Host-side spectral norm (eval-mode power iteration), float32."""
    w = np.asarray(w, np.float32)
    u = np.asarray(u, np.float32)
    v = u @ w
    v = v / max(np.float32(np.linalg.norm(v)), np.float32(EPS))
    u2 = v @ w.T
    u2 = u2 / max(np.float32(np.linalg.norm(u2)), np.float32(EPS))
    sv = np.float32((v @ w.T @ u2.T)[0, 0])
    return w / sv


def _strip_pe_self_waits(nc):
    """Remove S[PE]-waits from PE matmuls: PE->PE deps are ordered by the
    engine queue + FIFO PSUM write port, and matmuls only have one ISA
    wait slot."""
    import concourse.mybir as mybir

    for f in nc.m.functions:
        for blk in f.blocks:
            for inst in blk.instructions:
                if not isinstance(inst, mybir.InstMatmult):
                    continue
                si = inst.sync_info
                kept = [w for w in si.on_wait
                        if not (w.ant_name or "").startswith("PE_")]
                if len(kept) != len(si.on_wait):
                    si.on_wait = kept
                    inst.sync_info = si


def _build_nc():
    import concourse.bass as bass
    import concourse.mybir as mybir
    import concourse.tile as tile
    from concourse import bacc
    from concourse.masks import make_identity

    fp32 = mybir.dt.float32
    fp16 = mybir.dt.float16
    bf16 = mybir.dt.bfloat16
    fp8 = mybir.dt.float8e4
    DR = mybir.MatmulPerfMode.DoubleRow
    Exp = mybir.ActivationFunctionType.Exp
    mult = mybir.AluOpType.mult
    add = mybir.AluOpType.add
    mx = mybir.AluOpType.max

    nc = bacc.Bacc()
    # weights arrive host-pretransposed to [p, ...] layouts so their DMAs
    # are contiguous per-partition lines on the HARDWARE dynamic queues
    # (the gpsimd software queue starts ~4us late and paced the old head)
    x16_d = nc.dram_tensor("x16", [C, HW], fp16, kind="ExternalInput").ap()
    x8_d = nc.dram_tensor("x8", [C, HW], fp8, kind="ExternalInput").ap()
    wtp_d = nc.dram_tensor("wtp", [P, KC * P], fp16, kind="ExternalInput").ap()
    wg_d = nc.dram_tensor("wg", [P, KC * C2], fp8, kind="ExternalInput").ap()
    wo_d = nc.dram_tensor("wo", [P, 2 * C], fp8, kind="ExternalInput").ap()
    out_d = nc.dram_tensor("out", [C, HW], fp16, kind="ExternalOutput").ap()

    x16_r = x16_d.rearrange("(kc p) n -> p kc n", p=P)
    x8_r = x8_d.rearrange("(kc p) n -> p kc n", p=P)
    out_r = out_d.rearrange("(ig p) n -> p ig n", p=P)

    with tile.TileContext(nc) as tc:
        with tc.tile_pool(name="sb", bufs=1) as sb:
            # ---- persistent tiles ----
            x16 = sb.tile([P, KC, HW], fp16)
            x8 = sb.tile([P, KC, HW], fp8)
            wtp = sb.tile([P, KC, P], fp16)
            wg8 = sb.tile([P, KC, C2], fp8)
            wo8 = sb.tile([P, 2, C], fp8)
            theta_sb = sb.tile([P, HW], fp16)             # rows 64:128 duplicate
            phi2 = sb.tile([P, NB, 4, 32], fp16)          # rows 64:128 duplicate
            g2 = sb.tile([P, 2, M], bf16)                 # pooled, cg-major
            gT_sb = sb.tile([P, MC, C2], bf16)            # [m-part, mc, c]
            identity = sb.tile([P, P], bf16)
            ones_mat = sb.tile([P, P], bf16)

            phi_flat = phi2.rearrange("p a b c -> p (a b c)")
            g4 = g2.rearrange("p cg (fb h2 w2) -> p cg fb h2 w2", h2=4, w2=32)

            # ---- constants + HAM warmup ----
            nc.vector.memset(ones_mat, 1.0)
            with tc.tile_pool(name="psW", bufs=1, space="PSUM") as psW:
                # junk matmuls keep the PE busy during the DMA head so the
                # HAM clock-gate opens before the real matmul stream
                wps = psW.tile([P, P], fp32, tag="warm")
                for _ in range(20):
                    nc.tensor.matmul(wps, lhsT=ones_mat, rhs=ones_mat,
                                     start=True, stop=True)
            ident_raw = sb.tile([P, P], fp32)
            make_identity(nc, ident_raw)
            nc.scalar.copy(identity, ident_raw)

            # ---- input DMAs on the two hardware queues ----
            # Each queue sustains ~215GB/s (~430 combined).  Segments are
            # ordered earliest-deadline-first per queue: the projections
            # consume x16 columns (theta/phi) then the same x8 columns (g)
            # ~1.5us later, one 1024-col group per ~3.8us.  Weights lead.
            nc.sync.dma_start(wtp, wtp_d.rearrange("p (kc i) -> p kc i", kc=KC))
            nc.scalar.dma_start(wg8, wg_d.rearrange("p (kc i) -> p kc i", kc=KC))
            segs = [
                (x16, x16_r, 0, 256, nc.scalar),
                (x16, x16_r, 256, 512, nc.sync),
                (x16, x16_r, 512, 768, nc.scalar),
                (x16, x16_r, 768, 1024, nc.sync),
                (x8, x8_r, 0, 512, nc.sync),
                (x8, x8_r, 512, 1024, nc.scalar),
                (x16, x16_r, 1024, 1536, nc.sync),
                (x16, x16_r, 1536, 2048, nc.scalar),
                (x8, x8_r, 1024, 1536, nc.sync),
                (x8, x8_r, 1536, 2048, nc.scalar),
                (x16, x16_r, 2048, 2560, nc.sync),
                (x16, x16_r, 2560, 3072, nc.scalar),
                (x8, x8_r, 2048, 2560, nc.sync),
                (x8, x8_r, 2560, 3072, nc.scalar),
                (x16, x16_r, 3072, 3584, nc.sync),
                (x16, x16_r, 3584, 4096, nc.scalar),
                (x8, x8_r, 3072, 3584, nc.sync),
                (x8, x8_r, 3584, 4096, nc.scalar),
            ]
            for dst, src, lo, hi, eng in segs:
                eng.dma_start(dst[:, :, lo:hi], src[:, :, lo:hi])
            nc.scalar.dma_start(wo8, wo_d.rearrange("p (cg i) -> p cg i", cg=2))

            # ---------- projections ----------
            with (
                tc.tile_pool(name="psA", bufs=3, space="PSUM") as psA,
                tc.tile_pool(name="psT", bufs=2, space="PSUM") as psT,
            ):
                for fb2 in range(NB // 2):
                    # fused theta+phi projection: lhsT = [wt | wp] gives
                    # theta on out-partitions 0:64, phi on 64:128; both
                    # duplicated onto partitions 64:128 for sT row-packing
                    ps = psA.tile([P, 2, FB], fp32, tag="proj", name="ps")
                    for half in range(2):
                        for kc in range(KC):
                            nc.tensor.matmul(
                                ps[:, half, :],
                                lhsT=wtp[:, kc, :],
                                rhs=x16[:, kc, (2 * fb2 + half) * FB:(2 * fb2 + half + 1) * FB],
                                start=(kc == 0), stop=(kc == KC - 1),
                            )
                    th = ps[:C8].rearrange("p a b -> p (a b)")
                    thsl = slice(2 * fb2 * FB, (2 * fb2 + 2) * FB)
                    nc.scalar.copy(theta_sb[:C8, thsl], th)
                    # duplication for the sT row-packing rides gpsimd (idle
                    # during projections) instead of the busy DVE
                    nc.gpsimd.tensor_copy(theta_sb[C8:, thsl],
                                          theta_sb[:C8, thsl])
                    v = ps[C8:].rearrange("p fb (h2 hr w2 wr) -> p fb h2 hr w2 wr",
                                          hr=2, w2=32, wr=2)
                    dst = phi2[:C8, 2 * fb2:2 * fb2 + 2]   # [64, 2, 4, 32]
                    nc.vector.tensor_copy(dst, v[:, :, :, 0, :, 0])
                    nc.vector.tensor_tensor(dst, dst, v[:, :, :, 0, :, 1], mx)
                    nc.vector.tensor_tensor(dst, dst, v[:, :, :, 1, :, 0], mx)
                    nc.vector.tensor_tensor(dst, dst, v[:, :, :, 1, :, 1], mx)
                    nc.vector.tensor_copy(phi2[C8:, 2 * fb2:2 * fb2 + 2],
                                          phi2[:C8, 2 * fb2:2 * fb2 + 2])

                    # g projection + maxpool on the same x16 columns, then
                    # transpose this fb2's two m-chunks while they're hot
                    for cg in range(2):
                        ps = psA.tile([P, 2, FB], fp32, tag="proj", name="psg")
                        for half in range(2):
                            hsl = slice((2 * fb2 + half) * FB,
                                        (2 * fb2 + half + 1) * FB)
                            # fp8 DoubleRow: k=256 per matmul, halving the
                            # PSUM-write traffic of the g projection
                            for s in range(2):
                                nc.tensor.matmul(
                                    ps[:, half, :],
                                    lhsT=wg8[:, 2 * s:2 * s + 2, cg * P:(cg + 1) * P],
                                    rhs=x8[:, 2 * s:2 * s + 2, hsl],
                                    start=(s == 0), stop=(s == 1),
                                    perf_mode=DR,
                                )
                        v = ps.rearrange("p fb (h2 hr w2 wr) -> p fb h2 hr w2 wr",
                                         hr=2, w2=32, wr=2)
                        dst = g4[:, cg, 2 * fb2:2 * fb2 + 2]
                        nc.vector.tensor_copy(dst, v[:, :, :, 0, :, 0])
                        nc.vector.tensor_tensor(dst, dst, v[:, :, :, 0, :, 1], mx)
                        nc.vector.tensor_tensor(dst, dst, v[:, :, :, 1, :, 0], mx)
                        nc.vector.tensor_tensor(dst, dst, v[:, :, :, 1, :, 1], mx)
                    for mc in (2 * fb2, 2 * fb2 + 1):
                        pt = psT.tile([P, 2, P], bf16, tag="tr")
                        for cg in range(2):
                            nc.tensor.transpose(
                                pt[:, cg, :], g2[:, cg, mc * P:(mc + 1) * P],
                                identity,
                            )
                        nc.scalar.copy(gT_sb[:, mc, :],
                                       pt.rearrange("p a b -> p (a b)"))

            # ---------- attention ----------
            with (
                tc.tile_pool(name="psS", bufs=2, space="PSUM") as psS,
                tc.tile_pool(name="psO", bufs=2, space="PSUM") as psO,
                tc.tile_pool(name="psO2", bufs=2, space="PSUM") as psO2,
            ):
                pending = []

                def _emit_o2_early(item):
                    """Out-projection matmuls of block jnb (4 fp8-DR, k=256)
                    + the 2 DVE fused residuals. PE work lands early to fill
                    the exp-pipeline wait of the current block."""
                    jnb, jo8 = item
                    jsl = slice(jnb * FB, (jnb + 1) * FB)
                    ot4 = sb.tile([P, 4, FB], fp16, tag="out", bufs=2,
                                  name="ot4")
                    o2s = []
                    for ig in range(4):
                        o2 = psO2.tile([P, FB], fp32, tag="o2", name="o2")
                        nc.tensor.matmul(
                            o2,
                            lhsT=wo8[:, :, ig * P:(ig + 1) * P],
                            rhs=jo8,
                            start=True, stop=True, perf_mode=DR,
                        )
                        if ig < 2:
                            # DVE: (o2 * 1/WO_SCALE) + x16 in one op
                            nc.vector.scalar_tensor_tensor(
                                ot4[:, ig, :], o2, 1.0 / WO_SCALE,
                                x16[:, ig, jsl], op0=mult, op1=add)
                        else:
                            o2s.append(o2)
                    return (jnb, ot4, o2s)

                def _emit_o2_late(item):
                    """ACT copy-with-scale + GPSIMD residual adds for ig 2,3
                    and the single out-DMA; emitted after the sT pairs so
                    ACT's exp stream is not interrupted."""
                    jnb, ot4, o2s = item
                    jsl = slice(jnb * FB, (jnb + 1) * FB)
                    for ig, o2 in zip((2, 3), o2s):
                        st = sb.tile([P, FB], fp32, tag="stage", bufs=4,
                                     name="st")
                        nc.scalar.mul(st, o2, 1.0 / WO_SCALE)
                        nc.gpsimd.tensor_tensor(ot4[:, ig, :], st,
                                                x16[:, ig, jsl], add)
                    nc.sync.dma_start(out_r[:, :, jsl], ot4)

                for nb in range(NB):
                    nsl = slice(nb * FB, (nb + 1) * FB)
                    # sT[m, n] = sum_c phi[c, m] theta[c, n]: k=64 pairs
                    # run concurrently in disjoint PE row-halves
                    expT = sb.tile([P, MC, FB], bf16, tag="expT", bufs=3)

                    def _st_pair(mc2):
                        ps = psS.tile([P, 2, FB], fp32, tag="sT")
                        nc.tensor.matmul(
                            ps[:, 0, :],
                            lhsT=phi_flat[:C8, (2 * mc2) * P:(2 * mc2 + 1) * P],
                            rhs=theta_sb[:C8, nsl],
                            start=True, stop=True, tile_position=(0, 0),
                        )
                        nc.tensor.matmul(
                            ps[:, 1, :],
                            lhsT=phi_flat[C8:, (2 * mc2 + 1) * P:(2 * mc2 + 2) * P],
                            rhs=theta_sb[C8:, nsl],
                            start=True, stop=True, tile_position=(64, 0),
                        )
                        nc.scalar.activation(
                            expT[:, 2 * mc2:2 * mc2 + 2, :].rearrange("p a b -> p (a b)"),
                            ps.rearrange("p a b -> p (a b)"), Exp,
                        )

                    part = sb.tile([P, 4, FB], bf16, tag="part", bufs=2)
                    _st_pair(0)
                    _st_pair(1)
                    # previous block's out-proj matmuls fill the exp wait
                    held = _emit_o2_early(pending.pop(0)) if pending else None
                    _st_pair(2)
                    # sum over m, level 1: two DVE adds, each ready as
                    # soon as its pair of exps lands (a: exps 0+2,
                    # b: exps 1+3), then FOUR accumulating ones-matmuls
                    # finish the m-sum on the PE
                    nc.vector.tensor_tensor(
                        part[:, 0:2, :].rearrange("p a b -> p (a b)"),
                        expT[:, 0:2, :].rearrange("p a b -> p (a b)"),
                        expT[:, 4:6, :].rearrange("p a b -> p (a b)"), add)
                    _st_pair(3)
                    if held is not None:
                        _emit_o2_late(held)
                    nc.vector.tensor_tensor(
                        part[:, 2:4, :].rearrange("p a b -> p (a b)"),
                        expT[:, 2:4, :].rearrange("p a b -> p (a b)"),
                        expT[:, 6:8, :].rearrange("p a b -> p (a b)"), add)

                    # o[c, n] = sum_m gT[m, c] expT[m, n]; the sum-matmuls
                    # sit between the two cg halves so the recip is ready
                    # when the first half finishes accumulating
                    o8_sb = sb.tile([P, 2, FB], fp8, tag="o8", bufs=2)
                    o_ps0 = psO.tile([P, FB], fp32, tag="o_ps", name="o0")
                    for mc in range(MC):
                        nc.tensor.matmul(
                            o_ps0,
                            lhsT=gT_sb[:, mc, 0:P],
                            rhs=expT[:, mc, :],
                            start=(mc == 0), stop=(mc == MC - 1),
                        )
                    sum_ps = psS.tile([P, 2, FB], fp32, tag="sT",
                                      name="sum_ps")[:, 0, :]
                    for j in range(4):
                        nc.tensor.matmul(sum_ps, lhsT=ones_mat,
                                         rhs=part[:, j, :],
                                         start=(j == 0), stop=(j == 3))
                    recipb = sb.tile([P, FB], fp32, tag="recipb", bufs=2)
                    nc.vector.reciprocal_approx_fast(recipb, sum_ps)
                    o_ps1 = psO.tile([P, FB], fp32, tag="o_ps", name="o1")
                    for mc in range(MC):
                        nc.tensor.matmul(
                            o_ps1,
                            lhsT=gT_sb[:, mc, P:C2],
                            rhs=expT[:, mc, :],
                            start=(mc == 0), stop=(mc == MC - 1),
                        )
                    nc.vector.tensor_tensor(o8_sb[:, 0, :], o_ps0, recipb, mult)
                    nc.vector.tensor_tensor(o8_sb[:, 1, :], o_ps1, recipb, mult)

                    pending.append((nb, o8_sb))

                # final block: no next block's exps to protect, so all four
                # residuals ride the (now idle) DVE directly from PSUM and
                # the out-DMA is split per half so rows fly as they finish
                jnb, jo8 = pending.pop(0)
                jsl = slice(jnb * FB, (jnb + 1) * FB)
                ot4 = sb.tile([P, 4, FB], fp16, tag="out", bufs=2, name="ot4")
                for ig in range(4):
                    o2 = psO2.tile([P, FB], fp32, tag="o2", name="o2")
                    nc.tensor.matmul(
                        o2,
                        lhsT=wo8[:, :, ig * P:(ig + 1) * P],
                        rhs=jo8,
                        start=True, stop=True, perf_mode=DR,
                    )
                    nc.vector.scalar_tensor_tensor(
                        ot4[:, ig, :], o2, 1.0 / WO_SCALE,
                        x16[:, ig, jsl], op0=mult, op1=add)
                    if ig == 1:
                        nc.sync.dma_start(out_r[:, 0:2, jsl], ot4[:, 0:2, :])
                nc.sync.dma_start(out_r[:, 2:4, jsl], ot4[:, 2:4, :])

    _strip_pe_self_waits(nc)
    nc.compile()
    return nc


def _get_nc():
    if "nc" not in _CACHE:
        _CACHE["nc"] = _build_nc()
    return _CACHE["nc"]


def make_in_maps(x, w_theta, w_phi, w_g, w_o, u_theta, u_phi, u_g, u_o, gamma):
    import ml_dtypes

    e4 = ml_dtypes.float8_e4m3
    wt = _sn(w_theta, u_theta).T                                  # [512, 64]
    wp = _sn(w_phi, u_phi).T                                      # [512, 64]
    wtp = np.concatenate([wt, wp], axis=1).astype(np.float16)     # [512, 128]
    wg = _sn(w_g, u_g).T.astype(e4)                               # [512, 256]
    wo = (WO_SCALE * np.float32(np.asarray(gamma, np.float32))
          * _sn(w_o, u_o).T).astype(e4)                           # [256, 512]
    # pre-transpose to [p, kc, i] so the device DMA is a contiguous
    # per-partition copy on the hardware dynamic queues
    wtp = np.ascontiguousarray(
        wtp.reshape(KC, P, P).transpose(1, 0, 2).reshape(P, KC * P))
    wg = np.ascontiguousarray(
        wg.reshape(KC, P, C2).transpose(1, 0, 2).reshape(P, KC * C2))
    wo = np.ascontiguousarray(
        wo.reshape(2, P, C).transpose(1, 0, 2).reshape(P, 2 * C))
    xf = np.asarray(x, np.float32).reshape(B, C, HW)
    x16 = xf.astype(np.float16)
    x8 = xf.astype(e4)
    return [
        {"x16": np.ascontiguousarray(x16[i]),
         "x8": np.ascontiguousarray(x8[i]),
         "wtp": wtp, "wg": wg, "wo": wo}
        for i in range(B)
    ]


def kernel(x, w_theta, w_phi, w_g, w_o, u_theta, u_phi, u_g, u_o, gamma):
    from concourse.bass_utils import run_bass_kernel_spmd

    in_maps = make_in_maps(
        x, w_theta, w_phi, w_g, w_o, u_theta, u_phi, u_g, u_o, gamma
    )
    nc = _get_nc()
    res = run_bass_kernel_spmd(nc, in_maps, core_ids=list(range(B)))
    out = np.stack([np.asarray(r["out"], np.float32) for r in res.results],
                   axis=0)
    return out.reshape(B, C, H, W)

